# revision 67
# baseline (speedup 1.0000x reference)
"""Paged GQA decode attention on 8 TRN2 NeuronCores.

Sharding: tensor-parallel over heads. Core m owns kv head m and query
heads [4m, 4m+4). block_tables / slot_mapping are applied on the host,
which gathers each sequence's valid cache prefix (new k/v token
scattered in) into dense per-core layouts; context_lens are baked into
the (shared SPMD) graph as static loop bounds. No collectives.

Per-core HBM layout (host-prepared from the full inputs):
  qt  [128, 64]          bf16, qt[d, 4b+h] = q[b, 4m+h, d] * scale
  kt  [128, KTOT]        fp8 e3m4, K^T, sequences PACKED with no pad
                         columns (KTOT = sum of context lens)
  vt  [128, TTOT, 128]   fp8 e3m4, V in 128-token tiles, partition =
                         token-within-tile, free = head dim
Outputs (host finishes the softmax normalization + transpose):
  o   [128, 64]  f32     o[d, 4i+h] = unnormalized attn out, seq order i
  den [1, 4*TTOT] f32    per-tile softmax partial sums (ones-matmul)

Device, per sequence b with S = context_lens[b], nt = ceil(S/128):
  scoresT[s, 4t+h] via matmul(lhsT=K-tile fp8 [128d, T], rhs=qt_b bf16)
  one accumulation-style matmul group per sequence (disjoint PSUM
  ranges; non-stop matmuls skip the per-instruction drain barrier:
  ~65 -> ~35 ns per 128-token tile)
  exp on ScalarE (PSUM f32 -> SBUF bf16), garbage rows of the last
  partial tile pre-zeroed so the denominator matmul can contract all
  128 partitions
  den partials: matmul(lhsT=ones [128,1], rhs=pt [128, 4nt]) in phase
  A (copies on VectorE so the ps_d recycle chain never waits the exps)
  oT[128d, 4h] += matmul(lhsT=V-tile fp8 [T, 128], rhs=pt-tile [T, 4])

Schedule: CHUNK-MAJOR MERGED STREAM.  K is cut into column chunks
(ramp 1/2 then 4 KB per partition) and V into tile chunks (24-tile,
8-tile at the end); chunks alternate across the Sync HWDGE and GpSimd
SWDGE rings by a cumulative-time model, and each V chunk is emitted as
soon as K coverage passes the end of the sequence containing its last
tile (+1 chunk margin for the exp).  The PE work for each tile is
emitted right after the chunk containing it, so the in-order PE stream
consumes chunks in exactly the emission order -- immune to ring-rate
drift (the rings' competitive service rates vary +-15% run to run) --
and alternates scores and PV continuously with no phase transition:
after the final landing only one small chunk's PV remains.  The Scalar
engine carries no bulk descriptors (they would block the exp chain).
Output: ONE full-width o-wave at the end (o padded to 128 f32
columns = 512 B partition rows, the SDMA line-rate floor -- narrower
rows pay a read-modify-write penalty per descriptor); the pending dens
drain as soon as the scores are exhausted and the den writeback rides
Scalar (on a bulk ring its gen's sem-wait would stall every remaining
chunk's descriptor-gen behind it), so after the last PV only the
o-wave's gen + receipt remain.
"""

import numpy as np

B = 16
H = 32
HKV = 8
D = 128
BLOCK = 256
MAX_KV = 4096
N_CORES = 8
HPC = H // N_CORES  # query heads per core
SCALE = np.float32(1.0 / np.sqrt(D))

from ml_dtypes import bfloat16 as _bf16
from ml_dtypes import float8_e3m4 as _f8

_graph_cache: dict = {}


def _plan(context_lens):
    """Order sequences for pipelined per-seq DMA. Returns (order, nts,
    offs, ttot, offc, ktot): nts[b]=ceil(S/128), offs[b]=tile offset,
    offc[b]=packed K column offset."""
    nts = [max(1, -(-int(s) // 128)) for s in context_lens]
    asc = sorted(range(B), key=lambda b: nts[b])
    # two tiny sequences first (instant pipeline fill), descending
    # middle so the big sequences' long chains overlap the DMA stream,
    # two small ones last so only short chains remain after the final
    # DMA byte
    order = tuple(asc[0:2] + asc[:3:-1] + asc[2:4])
    offs = {}
    offc = {}
    off = 0
    c = 0
    for b in order:
        offs[b] = off
        off += nts[b]
        offc[b] = c
        c += max(int(context_lens[b]), 1)
    return order, tuple(nts), offs, off, offc, c


def _build(context_lens):
    import concourse.bacc as bacc
    import concourse.mybir as mybir
    import concourse.tile as tile

    f32 = mybir.dt.float32
    bf16 = mybir.dt.bfloat16
    f8 = mybir.dt.float8e3
    order, nts, offs, ttot, offc, ktot = _plan(context_lens)
    seqlen = {b: max(int(context_lens[b]), 1) for b in order}
    nc = bacc.Bacc(None, target_bir_lowering=False)

    qt_ext = nc.declare_dram_parameter("qt", [D, B * HPC], bf16, isOutput=False)
    kt_ext = nc.declare_dram_parameter("kt", [D, ktot], f8, isOutput=False)
    vt_ext = nc.declare_dram_parameter("vt", [128, ttot, D], f8, isOutput=False)
    # o is padded to 128 f32 columns: 512 B per partition row, the
    # SDMA line-rate floor -- narrower rows pay a read-modify-write
    # penalty per descriptor (~1.5us on the final wave)
    o_ext = nc.declare_dram_parameter("o", [D, 2 * B * HPC], f32, isOutput=True)
    den_ext = nc.declare_dram_parameter("den", [1, HPC * ttot], f32, isOutput=True)

    with tile.TileContext(nc) as tc:
        with (
            tc.tile_pool(name="const", bufs=1) as const_pool,
            tc.tile_pool(name="ps_s", bufs=4, space="PSUM") as ps_s_pool,
            tc.tile_pool(name="ps_o", bufs=2, space="PSUM") as ps_o_pool,
            tc.tile_pool(name="ps_d", bufs=2, space="PSUM") as ps_d_pool,
        ):
            qt = const_pool.tile([D, B * HPC], bf16)
            # qt rides the Scalar ring: its 128 B rows pay the sub-512B
            # RMW penalty, and at the head of the Sync FIFO that would
            # delay the first K chunk's transfer behind it
            nc.scalar.dma_start(qt[:], qt_ext[:])
            ones = const_pool.tile([128, 1], bf16)
            nc.vector.memset(ones[:], 1.0)
            o_all = const_pool.tile([D, 2 * B * HPC], f32)
            den_all = const_pool.tile([1, HPC * ttot], f32)

            kt_all = const_pool.tile([D, ktot], f8)
            vt_all = const_pool.tile([128, ttot, D], f8)
            pts = {}
            for b in order:
                nt = nts[b]
                pts[b] = const_pool.tile([128, HPC * nt], bf16, name=f"pt{b}")
                T = seqlen[b] - (nt - 1) * 128
                if T < 128:
                    # zero the last partial tile's columns so the
                    # ones-matmul can contract all 128 partitions; the
                    # exp later overwrites rows [0:T] with valid values
                    nc.vector.memset(pts[b][:, HPC * nt - HPC : HPC * nt], 0.0)

            # ring model only balances BYTES between the rings; the
            # consumption order is emission order by construction
            qtime = [0.0, 0.0]  # 0 = gpsimd, 1 = sync
            rate = (175.0, 140.0)

            def assign(nbytes):
                kb = nbytes / 1000.0
                r = 0 if qtime[0] + kb / rate[0] <= qtime[1] + kb / rate[1] else 1
                qtime[r] += kb / rate[r]
                return nc.gpsimd if r == 0 else nc.sync

            # K chunk cuts: ramp in (PE starts within ~1us of the
            # stream) then 4 KB/partition (0.5 MB) steady
            k_cuts = [0]
            pos, ramp = 0, [1024, 2048]
            while pos < ktot:
                step = ramp.pop(0) if ramp else 4096
                pos = min(pos + step, ktot)
                k_cuts.append(pos)
            # V chunk cuts in tiles: 24-tile (0.375 MB) steady, 8-tile
            # for the last 32 so the final landings gate only a small
            # PV backlog (the 512 B/row SDMA line-rate floor rules out
            # finer chunks -- a V tile is only 128 B per partition row)
            v_cuts = [0]
            pos = 0
            while pos < ttot:
                left = ttot - pos
                if left <= 8:
                    step = 4  # 512 B rows: the SDMA line-rate floor
                elif left <= 32:
                    step = 8
                else:
                    step = 16
                pos = min(pos + step, ttot)
                v_cuts.append(pos)

            # merged K/V stream: each V chunk is emitted as soon as K
            # coverage passes the end of the sequence containing its
            # last tile (+1 chunk of margin for the exp) -- the PE
            # alternates scores and PV work continuously instead of a
            # K phase then a V phase, so no PV backlog accumulates and
            # after the final landing only one small chunk's PV remains
            tile_of = []
            for b in order:
                tile_of += [b] * nts[b]
            plan = []
            k_i = 1
            cov = 0
            for v_i in range(1, len(v_cuts)):
                hi_t = v_cuts[v_i]
                b_hi = tile_of[hi_t - 1]
                req = offc[b_hi] + seqlen[b_hi]
                while k_i < len(k_cuts) and cov < req:
                    plan.append(("K", k_cuts[k_i - 1], k_cuts[k_i]))
                    cov = k_cuts[k_i]
                    k_i += 1
                if k_i < len(k_cuts) and cov < req + 4096:
                    plan.append(("K", k_cuts[k_i - 1], k_cuts[k_i]))
                    cov = k_cuts[k_i]
                    k_i += 1
                plan.append(("V", v_cuts[v_i - 1], hi_t))
            while k_i < len(k_cuts):
                plan.append(("K", k_cuts[k_i - 1], k_cuts[k_i]))
                k_i += 1

            # ---- phase A: chunk-major scores + exp + den ----
            ps_s_of = {}

            def scores_tile(b, t):
                nt = nts[b]
                S = seqlen[b]
                if t == 0:
                    ps_s_of[b] = ps_s_pool.tile([128, 128], mybir.dt.float32, tag="s", name=f"ps_s{b}")
                T = min(128, S - t * 128)
                c0 = offc[b] + t * 128
                nc.tensor.matmul(
                    ps_s_of[b][0:T, HPC * t : HPC * t + HPC],
                    kt_all[:, c0 : c0 + T],
                    qt[:, HPC * b : HPC * b + HPC],
                    start=(t == 0),
                    stop=(t == nt - 1),
                )

            def exp_emit(b):
                S = seqlen[b]
                nt = nts[b]
                pt = pts[b]
                ps_s = ps_s_of[b]
                T = S - (nt - 1) * 128
                if T < 128:
                    if nt > 1:
                        nc.scalar.activation(
                            pt[:, 0 : HPC * (nt - 1)],
                            ps_s[:, 0 : HPC * (nt - 1)],
                            mybir.ActivationFunctionType.Exp,
                        )
                    nc.scalar.activation(
                        pt[0:T, HPC * (nt - 1) : HPC * nt],
                        ps_s[0:T, HPC * (nt - 1) : HPC * nt],
                        mybir.ActivationFunctionType.Exp,
                    )
                else:
                    nc.scalar.activation(
                        pt[:, 0 : HPC * nt],
                        ps_s[:, 0 : HPC * nt],
                        mybir.ActivationFunctionType.Exp,
                    )

            def den_emit(b):
                nt = nts[b]
                off = offs[b]
                ps_d = ps_d_pool.tile([1, 128], mybir.dt.float32, tag="d")
                nc.tensor.matmul(
                    ps_d[0:1, 0 : HPC * nt],
                    ones[:, 0:1],
                    pts[b][:, 0 : HPC * nt],
                    start=True,
                    stop=True,
                )
                nc.vector.tensor_copy(
                    den_all[0:1, HPC * off : HPC * off + HPC * nt],
                    ps_d[0:1, 0 : HPC * nt],
                )

            ps_o_of = {}

            def pv_tile(b, t, j):
                nt = nts[b]
                if t == 0:
                    ps_o_of[b] = ps_o_pool.tile(
                        [D, HPC], mybir.dt.float32, tag="o", name=f"ps_o{b}"
                    )
                T = min(128, seqlen[b] - t * 128)
                nc.tensor.matmul(
                    ps_o_of[b][:, :],
                    vt_all[0:T, offs[b] + t, :],
                    pts[b][0:T, HPC * t : HPC * t + HPC],
                    start=(t == 0),
                    stop=(t == nt - 1),
                )
                if t == nt - 1:
                    nc.vector.tensor_copy(
                        o_all[:, HPC * j : HPC * j + HPC], ps_o_of[b][:, :]
                    )

            # single merged walk over the plan: K chunks emit their
            # scores (exp when a sequence completes, dens pipelined
            # two behind), V chunks emit their PVs + o copies.  Output
            # waves: each o DMA costs a ~600ns DIRECT2D gen (128
            # descriptors, one per partition, regardless of column
            # count), so one overlapped wave + one final wave.  The
            # den writeback splits: a hidden mid-stream wave on Scalar
            # and a tiny end wave on GpSimd (idle by then), whose gen
            # runs in parallel with the final o-wave gen on Scalar.
            tiles_a = [(b, t) for b in order for t in range(nts[b])]
            jmap = {b: j for j, b in enumerate(order)}
            den_q = []
            fin = 0
            sti = 0
            pti = 0
            for kind, lo, hi in plan:
                if kind == "K":
                    assign((hi - lo) * 128).dma_start(
                        kt_all[:, lo:hi], kt_ext[:, lo:hi]
                    )
                    while sti < len(tiles_a):
                        b, t = tiles_a[sti]
                        T = min(128, seqlen[b] - t * 128)
                        if offc[b] + t * 128 + T > hi:
                            break
                        scores_tile(b, t)
                        if t == nts[b] - 1:
                            exp_emit(b)
                            den_q.append(b)
                            if len(den_q) >= 3:
                                den_emit(den_q.pop(0))
                        sti += 1
                else:
                    assign((hi - lo) * 128 * 128).dma_start(
                        vt_all[:, lo:hi, :], vt_ext[:, lo:hi, :]
                    )
                    if sti == len(tiles_a) and den_q:
                        # all scores emitted: drain the pending dens
                        # NOW (not after the walk) and ship the den
                        # writeback on SCALAR -- on gpsimd its gen's
                        # sem-wait (den copies finish only when the PE
                        # reaches the den matmuls) blocks the
                        # sequencer and stalls every remaining V
                        # chunk's descriptor-gen behind it by ~2us
                        while den_q:
                            den_emit(den_q.pop(0))
                        nc.scalar.dma_start(den_ext[:], den_all[:])
                    while pti < len(tiles_a):
                        b, t = tiles_a[pti]
                        if offs[b] + t >= hi:
                            break
                        pv_tile(b, t, jmap[b])
                        pti += 1
            while den_q:
                den_emit(den_q.pop(0))
                if not den_q:
                    nc.scalar.dma_start(den_ext[:], den_all[:])

            # ONE full-width o wave: 512 B rows (no RMW), one gen
            nc.scalar.dma_start(o_ext[:], o_all[:])

    nc.compile()
    return nc, order, nts, offs, ttot, offc, ktot


def _prep_inputs(inputs, order, nts, offs, ttot, offc, ktot):
    q = np.asarray(inputs["q"], dtype=np.float32)
    k = np.asarray(inputs["k"], dtype=np.float32)
    v = np.asarray(inputs["v"], dtype=np.float32)
    k_cache = np.asarray(inputs["k_cache"], dtype=np.float32)
    v_cache = np.asarray(inputs["v_cache"], dtype=np.float32)
    context_lens = np.asarray(inputs["context_lens"])
    block_tables = np.asarray(inputs["block_tables"])
    slot_mapping = np.asarray(inputs["slot_mapping"])
    nslot = k_cache.shape[0] * k_cache.shape[1]

    # per-seq gathered slot indices (ceil128 of context), block_tables applied
    slot_idx = {}
    for b in range(B):
        ncols = nts[b] * 128
        nblk = -(-ncols // BLOCK)
        blocks = block_tables[b, :nblk].astype(np.int64)
        idx = (blocks[:, None] * BLOCK + np.arange(BLOCK)[None, :]).reshape(-1)[:ncols]
        slot_idx[b] = idx

    in_maps = []
    for m in range(N_CORES):
        kc = k_cache[:, :, m, :].reshape(nslot, D)  # strided view
        vc = v_cache[:, :, m, :].reshape(nslot, D)
        kt = np.empty((D, ktot), dtype=_f8)
        vt = np.empty((128, ttot, D), dtype=_f8)
        for b in range(B):
            idx = slot_idx[b]
            kg = kc[idx]  # [ncols, 128] gather (copy)
            vg = vc[idx]
            # scatter the new token (reference's _store_kvcache)
            sm = int(slot_mapping[b])
            if sm >= 0:
                pos = np.nonzero(idx == sm)[0]
                if pos.size:
                    kg[pos[0]] = k[b, m]
                    vg[pos[0]] = v[b, m]
            off = offs[b]
            nt = nts[b]
            S = max(int(context_lens[b]), 1)
            kt[:, offc[b] : offc[b] + S] = kg[:S].T.astype(_f8)
            vt[:, off : off + nt, :] = (
                vg.reshape(nt, 128, D).transpose(1, 0, 2).astype(_f8)
            )
        qt = np.ascontiguousarray(
            (q[:, HPC * m : HPC * m + HPC, :].reshape(B * HPC, D) * SCALE).T
        ).astype(_bf16)
        in_maps.append({"qt": qt, "kt": kt, "vt": vt})
    return in_maps


def _run(inputs: dict, trace: bool = False, tmpdir: str | None = None):
    from concourse.bass_utils import run_bass_kernel_spmd

    context_lens = np.asarray(inputs["context_lens"])
    key = tuple(int(x) for x in context_lens)
    cached = _graph_cache.get(key)
    if cached is None:
        cached = _build(context_lens)
        _graph_cache[key] = cached
    nc, order, nts, offs, ttot, offc, ktot = cached

    in_maps = _prep_inputs(inputs, order, nts, offs, ttot, offc, ktot)
    res = run_bass_kernel_spmd(
        nc, in_maps, list(range(N_CORES)), trace=trace, tmpdir=tmpdir
    )

    out = np.empty((B, 1, H, D), dtype=np.float32)
    for m in range(N_CORES):
        om = np.asarray(res.results[m]["o"])  # [128, 64] f32, o^T
        dm = np.asarray(res.results[m]["den"]).reshape(-1)  # [4*ttot]
        for i, b in enumerate(order):
            off = offs[b]
            nt = nts[b]
            den = dm[HPC * off : HPC * off + HPC * nt].reshape(nt, HPC).sum(axis=0)
            oT = om[:, HPC * i : HPC * i + HPC]  # [128, 4]
            out[b, 0, HPC * m : HPC * m + HPC, :] = (oT / den[None, :]).T
    return out, res


def kernel(**inputs) -> np.ndarray:
    out, _ = _run(inputs, trace=False)
    return out


# revision 68
# speedup vs baseline: 1.0123x; 1.0123x over previous
"""Paged GQA decode attention on 8 TRN2 NeuronCores.

Sharding: tensor-parallel over heads. Core m owns kv head m and query
heads [4m, 4m+4). block_tables / slot_mapping are applied on the host,
which gathers each sequence's valid cache prefix (new k/v token
scattered in) into dense per-core layouts; context_lens are baked into
the (shared SPMD) graph as static loop bounds. No collectives.

Per-core HBM layout (host-prepared from the full inputs):
  qt  [128, 64]          bf16, qt[d, 4b+h] = q[b, 4m+h, d] * scale
  kt  [128, KTOT]        fp8 e3m4, K^T, sequences PACKED with no pad
                         columns (KTOT = sum of context lens)
  vt  [128, TTOT, 128]   fp8 e3m4, V in 128-token tiles, partition =
                         token-within-tile, free = head dim
Outputs (host finishes the softmax normalization + transpose):
  o   [128, 64]  f32     o[d, 4i+h] = unnormalized attn out, seq order i
  den [1, 4*TTOT] f32    per-tile softmax partial sums (ones-matmul)

Device, per sequence b with S = context_lens[b], nt = ceil(S/128):
  scoresT[s, 4t+h] via matmul(lhsT=K-tile fp8 [128d, T], rhs=qt_b bf16)
  one accumulation-style matmul group per sequence (disjoint PSUM
  ranges; non-stop matmuls skip the per-instruction drain barrier:
  ~65 -> ~35 ns per 128-token tile)
  exp on ScalarE (PSUM f32 -> SBUF bf16), garbage rows of the last
  partial tile pre-zeroed so the denominator matmul can contract all
  128 partitions
  den partials: matmul(lhsT=ones [128,1], rhs=pt [128, 4nt]) in phase
  A (copies on VectorE so the ps_d recycle chain never waits the exps)
  oT[128d, 4h] += matmul(lhsT=V-tile fp8 [T, 128], rhs=pt-tile [T, 4])

Schedule: CHUNK-MAJOR MERGED STREAM.  K is cut into column chunks
(ramp 1/2 then 4 KB per partition) and V into tile chunks (24-tile,
8-tile at the end); chunks alternate across the Sync HWDGE and GpSimd
SWDGE rings by a cumulative-time model, and each V chunk is emitted as
soon as K coverage passes the end of the sequence containing its last
tile (+1 chunk margin for the exp).  The PE work for each tile is
emitted right after the chunk containing it, so the in-order PE stream
consumes chunks in exactly the emission order -- immune to ring-rate
drift (the rings' competitive service rates vary +-15% run to run) --
and alternates scores and PV continuously with no phase transition:
after the final landing only one small chunk's PV remains.  The Scalar
engine carries no bulk descriptors (they would block the exp chain).
Output: ONE full-width o-wave at the end (o padded to 128 f32
columns = 512 B partition rows, the SDMA line-rate floor -- narrower
rows pay a read-modify-write penalty per descriptor); the pending dens
drain as soon as the scores are exhausted and the den writeback rides
Scalar (on a bulk ring its gen's sem-wait would stall every remaining
chunk's descriptor-gen behind it), so after the last PV only the
o-wave's gen + receipt remain.
"""

import numpy as np

B = 16
H = 32
HKV = 8
D = 128
BLOCK = 256
MAX_KV = 4096
N_CORES = 8
HPC = H // N_CORES  # query heads per core
SCALE = np.float32(1.0 / np.sqrt(D))

from ml_dtypes import bfloat16 as _bf16
from ml_dtypes import float8_e3m4 as _f8

_graph_cache: dict = {}


def _plan(context_lens):
    """Order sequences for pipelined per-seq DMA. Returns (order, nts,
    offs, ttot, offc, ktot): nts[b]=ceil(S/128), offs[b]=tile offset,
    offc[b]=packed K column offset."""
    nts = [max(1, -(-int(s) // 128)) for s in context_lens]
    asc = sorted(range(B), key=lambda b: nts[b])
    # two tiny sequences first (instant pipeline fill), descending
    # middle so the big sequences' long chains overlap the DMA stream,
    # two small ones last so only short chains remain after the final
    # DMA byte
    order = tuple(asc[0:2] + asc[:3:-1] + asc[2:4])
    offs = {}
    offc = {}
    off = 0
    c = 0
    for b in order:
        offs[b] = off
        off += nts[b]
        offc[b] = c
        c += max(int(context_lens[b]), 1)
    return order, tuple(nts), offs, off, offc, c


def _build(context_lens):
    import concourse.bacc as bacc
    import concourse.mybir as mybir
    import concourse.tile as tile

    f32 = mybir.dt.float32
    bf16 = mybir.dt.bfloat16
    f8 = mybir.dt.float8e3
    order, nts, offs, ttot, offc, ktot = _plan(context_lens)
    seqlen = {b: max(int(context_lens[b]), 1) for b in order}
    nc = bacc.Bacc(None, target_bir_lowering=False)

    qt_ext = nc.declare_dram_parameter("qt", [D, B * HPC], bf16, isOutput=False)
    kt_ext = nc.declare_dram_parameter("kt", [D, ktot], f8, isOutput=False)
    vt_ext = nc.declare_dram_parameter("vt", [128, ttot, D], f8, isOutput=False)
    # o is padded to 128 f32 columns: 512 B per partition row, the
    # SDMA line-rate floor -- narrower rows pay a read-modify-write
    # penalty per descriptor (~1.5us on the final wave)
    o_ext = nc.declare_dram_parameter("o", [D, 2 * B * HPC], f32, isOutput=True)
    den_ext = nc.declare_dram_parameter("den", [1, HPC * ttot], f32, isOutput=True)

    with tile.TileContext(nc) as tc:
        with (
            tc.tile_pool(name="const", bufs=1) as const_pool,
            tc.tile_pool(name="ps_s", bufs=3, space="PSUM") as ps_s_pool,
            tc.tile_pool(name="ps_o", bufs=4, space="PSUM") as ps_o_pool,
            tc.tile_pool(name="ps_d", bufs=1, space="PSUM") as ps_d_pool,
        ):
            qt = const_pool.tile([D, B * HPC], bf16)
            # qt rides the Scalar ring: its 128 B rows pay the sub-512B
            # RMW penalty, and at the head of the Sync FIFO that would
            # delay the first K chunk's transfer behind it
            nc.scalar.dma_start(qt[:], qt_ext[:])
            ones = const_pool.tile([128, 1], bf16)
            nc.vector.memset(ones[:], 1.0)
            o_all = const_pool.tile([D, 2 * B * HPC], f32)
            den_all = const_pool.tile([1, HPC * ttot], f32)

            kt_all = const_pool.tile([D, ktot], f8)
            vt_all = const_pool.tile([128, ttot, D], f8)
            pts = {}
            for b in order:
                nt = nts[b]
                pts[b] = const_pool.tile([128, HPC * nt], bf16, name=f"pt{b}")
                T = seqlen[b] - (nt - 1) * 128
                if T < 128:
                    # zero the last partial tile's columns so the
                    # ones-matmul can contract all 128 partitions; the
                    # exp later overwrites rows [0:T] with valid values
                    nc.vector.memset(pts[b][:, HPC * nt - HPC : HPC * nt], 0.0)

            # ring model only balances BYTES between the rings; the
            # consumption order is emission order by construction
            qtime = [0.0, 0.0]  # 0 = gpsimd, 1 = sync
            rate = (175.0, 140.0)

            def assign(nbytes):
                kb = nbytes / 1000.0
                r = 0 if qtime[0] + kb / rate[0] <= qtime[1] + kb / rate[1] else 1
                qtime[r] += kb / rate[r]
                return nc.gpsimd if r == 0 else nc.sync

            # K chunk cuts: ramp in (PE starts within ~1us of the
            # stream) then 4 KB/partition (0.5 MB) steady
            k_cuts = [0]
            pos, ramp = 0, [1024, 2048]
            while pos < ktot:
                step = ramp.pop(0) if ramp else 4096
                pos = min(pos + step, ktot)
                k_cuts.append(pos)
            # V chunk cuts in tiles: 24-tile (0.375 MB) steady, 8-tile
            # for the last 32 so the final landings gate only a small
            # PV backlog (the 512 B/row SDMA line-rate floor rules out
            # finer chunks -- a V tile is only 128 B per partition row)
            v_cuts = [0]
            pos = 0
            while pos < ttot:
                left = ttot - pos
                if left <= 8:
                    step = 4  # 512 B rows: the SDMA line-rate floor
                elif left <= 32:
                    step = 8
                else:
                    step = 16
                pos = min(pos + step, ttot)
                v_cuts.append(pos)

            # merged K/V stream: each V chunk is emitted as soon as K
            # coverage passes the end of the sequence containing its
            # last tile (+1 chunk of margin for the exp) -- the PE
            # alternates scores and PV work continuously instead of a
            # K phase then a V phase, so no PV backlog accumulates and
            # after the final landing only one small chunk's PV remains
            tile_of = []
            for b in order:
                tile_of += [b] * nts[b]
            plan = []
            k_i = 1
            cov = 0
            for v_i in range(1, len(v_cuts)):
                hi_t = v_cuts[v_i]
                b_hi = tile_of[hi_t - 1]
                req = offc[b_hi] + seqlen[b_hi]
                while k_i < len(k_cuts) and cov < req:
                    plan.append(("K", k_cuts[k_i - 1], k_cuts[k_i]))
                    cov = k_cuts[k_i]
                    k_i += 1
                if k_i < len(k_cuts) and cov < req + 4096:
                    plan.append(("K", k_cuts[k_i - 1], k_cuts[k_i]))
                    cov = k_cuts[k_i]
                    k_i += 1
                plan.append(("V", v_cuts[v_i - 1], hi_t))
            while k_i < len(k_cuts):
                plan.append(("K", k_cuts[k_i - 1], k_cuts[k_i]))
                k_i += 1

            # ---- phase A: chunk-major scores + exp + den ----
            ps_s_of = {}

            def scores_tile(b, t):
                nt = nts[b]
                S = seqlen[b]
                if t == 0:
                    ps_s_of[b] = ps_s_pool.tile([128, 128], mybir.dt.float32, tag="s", name=f"ps_s{b}")
                T = min(128, S - t * 128)
                c0 = offc[b] + t * 128
                nc.tensor.matmul(
                    ps_s_of[b][0:T, HPC * t : HPC * t + HPC],
                    kt_all[:, c0 : c0 + T],
                    qt[:, HPC * b : HPC * b + HPC],
                    start=(t == 0),
                    stop=(t == nt - 1),
                )

            def exp_emit(b):
                S = seqlen[b]
                nt = nts[b]
                pt = pts[b]
                ps_s = ps_s_of[b]
                T = S - (nt - 1) * 128
                if T < 128:
                    if nt > 1:
                        nc.scalar.activation(
                            pt[:, 0 : HPC * (nt - 1)],
                            ps_s[:, 0 : HPC * (nt - 1)],
                            mybir.ActivationFunctionType.Exp,
                        )
                    nc.scalar.activation(
                        pt[0:T, HPC * (nt - 1) : HPC * nt],
                        ps_s[0:T, HPC * (nt - 1) : HPC * nt],
                        mybir.ActivationFunctionType.Exp,
                    )
                else:
                    nc.scalar.activation(
                        pt[:, 0 : HPC * nt],
                        ps_s[:, 0 : HPC * nt],
                        mybir.ActivationFunctionType.Exp,
                    )

            def den_emit(b):
                nt = nts[b]
                off = offs[b]
                ps_d = ps_d_pool.tile([1, 128], mybir.dt.float32, tag="d")
                nc.tensor.matmul(
                    ps_d[0:1, 0 : HPC * nt],
                    ones[:, 0:1],
                    pts[b][:, 0 : HPC * nt],
                    start=True,
                    stop=True,
                )
                nc.vector.tensor_copy(
                    den_all[0:1, HPC * off : HPC * off + HPC * nt],
                    ps_d[0:1, 0 : HPC * nt],
                )

            ps_o_of = {}

            def pv_tile(b, t, j):
                # PV accumulation alternates between TWO PSUM tiles:
                # back-to-back accumulating matmuls to the SAME address
                # serialize on the drain (read-add-write), ~12 ns/tile
                # vs the scores' disjoint-column writes.  The combine
                # stages pb through o_all's padding half (scratch) so
                # the vector add reads PSUM+SBUF, then writes the sum.
                nt = nts[b]
                if t == 0:
                    pa = ps_o_pool.tile(
                        [D, HPC], mybir.dt.float32, tag="o", name=f"ps_oa{b}"
                    )
                    pb = (
                        ps_o_pool.tile(
                            [D, HPC], mybir.dt.float32, tag="o", name=f"ps_ob{b}"
                        )
                        if nt > 1
                        else None
                    )
                    ps_o_of[b] = (pa, pb)
                pa, pb = ps_o_of[b]
                acc = pa if t % 2 == 0 else pb
                last_same = nt - 1 if (nt - 1) % 2 == t % 2 else nt - 2
                T = min(128, seqlen[b] - t * 128)
                nc.tensor.matmul(
                    acc[:, :],
                    vt_all[0:T, offs[b] + t, :],
                    pts[b][0:T, HPC * t : HPC * t + HPC],
                    start=(t <= 1),
                    stop=(t == last_same),
                )
                if t == nt - 1:
                    dst = o_all[:, HPC * j : HPC * j + HPC]
                    if pb is None:
                        nc.vector.tensor_copy(dst, pa[:, :])
                    else:
                        pad = o_all[
                            :, B * HPC + HPC * j : B * HPC + HPC * j + HPC
                        ]
                        nc.vector.tensor_copy(pad, pb[:, :])
                        nc.vector.scalar_tensor_tensor(
                            dst,
                            pa[:, :],
                            1.0,
                            pad,
                            mybir.AluOpType.mult,
                            mybir.AluOpType.add,
                        )

            # single merged walk over the plan: K chunks emit their
            # scores (exp when a sequence completes, dens pipelined
            # two behind), V chunks emit their PVs + o copies.  Output
            # waves: each o DMA costs a ~600ns DIRECT2D gen (128
            # descriptors, one per partition, regardless of column
            # count), so one overlapped wave + one final wave.  The
            # den writeback splits: a hidden mid-stream wave on Scalar
            # and a tiny end wave on GpSimd (idle by then), whose gen
            # runs in parallel with the final o-wave gen on Scalar.
            tiles_a = [(b, t) for b in order for t in range(nts[b])]
            jmap = {b: j for j, b in enumerate(order)}
            den_q = []
            fin = 0
            sti = 0
            pti = 0
            for kind, lo, hi in plan:
                if kind == "K":
                    assign((hi - lo) * 128).dma_start(
                        kt_all[:, lo:hi], kt_ext[:, lo:hi]
                    )
                    while sti < len(tiles_a):
                        b, t = tiles_a[sti]
                        T = min(128, seqlen[b] - t * 128)
                        if offc[b] + t * 128 + T > hi:
                            break
                        scores_tile(b, t)
                        if t == nts[b] - 1:
                            exp_emit(b)
                            den_q.append(b)
                            if len(den_q) >= 3:
                                den_emit(den_q.pop(0))
                        sti += 1
                else:
                    assign((hi - lo) * 128 * 128).dma_start(
                        vt_all[:, lo:hi, :], vt_ext[:, lo:hi, :]
                    )
                    if sti == len(tiles_a) and den_q:
                        # all scores emitted: drain the pending dens
                        # NOW (not after the walk) and ship the den
                        # writeback on SCALAR -- on gpsimd its gen's
                        # sem-wait (den copies finish only when the PE
                        # reaches the den matmuls) blocks the
                        # sequencer and stalls every remaining V
                        # chunk's descriptor-gen behind it by ~2us
                        while den_q:
                            den_emit(den_q.pop(0))
                        nc.scalar.dma_start(den_ext[:], den_all[:])
                    while pti < len(tiles_a):
                        b, t = tiles_a[pti]
                        if offs[b] + t >= hi:
                            break
                        pv_tile(b, t, jmap[b])
                        pti += 1
            while den_q:
                den_emit(den_q.pop(0))
                if not den_q:
                    nc.scalar.dma_start(den_ext[:], den_all[:])

            # ONE full-width o wave: 512 B rows (no RMW), one gen
            nc.scalar.dma_start(o_ext[:], o_all[:])

    nc.compile()
    return nc, order, nts, offs, ttot, offc, ktot


def _prep_inputs(inputs, order, nts, offs, ttot, offc, ktot):
    q = np.asarray(inputs["q"], dtype=np.float32)
    k = np.asarray(inputs["k"], dtype=np.float32)
    v = np.asarray(inputs["v"], dtype=np.float32)
    k_cache = np.asarray(inputs["k_cache"], dtype=np.float32)
    v_cache = np.asarray(inputs["v_cache"], dtype=np.float32)
    context_lens = np.asarray(inputs["context_lens"])
    block_tables = np.asarray(inputs["block_tables"])
    slot_mapping = np.asarray(inputs["slot_mapping"])
    nslot = k_cache.shape[0] * k_cache.shape[1]

    # per-seq gathered slot indices (ceil128 of context), block_tables applied
    slot_idx = {}
    for b in range(B):
        ncols = nts[b] * 128
        nblk = -(-ncols // BLOCK)
        blocks = block_tables[b, :nblk].astype(np.int64)
        idx = (blocks[:, None] * BLOCK + np.arange(BLOCK)[None, :]).reshape(-1)[:ncols]
        slot_idx[b] = idx

    in_maps = []
    for m in range(N_CORES):
        kc = k_cache[:, :, m, :].reshape(nslot, D)  # strided view
        vc = v_cache[:, :, m, :].reshape(nslot, D)
        kt = np.empty((D, ktot), dtype=_f8)
        vt = np.empty((128, ttot, D), dtype=_f8)
        for b in range(B):
            idx = slot_idx[b]
            kg = kc[idx]  # [ncols, 128] gather (copy)
            vg = vc[idx]
            # scatter the new token (reference's _store_kvcache)
            sm = int(slot_mapping[b])
            if sm >= 0:
                pos = np.nonzero(idx == sm)[0]
                if pos.size:
                    kg[pos[0]] = k[b, m]
                    vg[pos[0]] = v[b, m]
            off = offs[b]
            nt = nts[b]
            S = max(int(context_lens[b]), 1)
            kt[:, offc[b] : offc[b] + S] = kg[:S].T.astype(_f8)
            vt[:, off : off + nt, :] = (
                vg.reshape(nt, 128, D).transpose(1, 0, 2).astype(_f8)
            )
        qt = np.ascontiguousarray(
            (q[:, HPC * m : HPC * m + HPC, :].reshape(B * HPC, D) * SCALE).T
        ).astype(_bf16)
        in_maps.append({"qt": qt, "kt": kt, "vt": vt})
    return in_maps


def _run(inputs: dict, trace: bool = False, tmpdir: str | None = None):
    from concourse.bass_utils import run_bass_kernel_spmd

    context_lens = np.asarray(inputs["context_lens"])
    key = tuple(int(x) for x in context_lens)
    cached = _graph_cache.get(key)
    if cached is None:
        cached = _build(context_lens)
        _graph_cache[key] = cached
    nc, order, nts, offs, ttot, offc, ktot = cached

    in_maps = _prep_inputs(inputs, order, nts, offs, ttot, offc, ktot)
    res = run_bass_kernel_spmd(
        nc, in_maps, list(range(N_CORES)), trace=trace, tmpdir=tmpdir
    )

    out = np.empty((B, 1, H, D), dtype=np.float32)
    for m in range(N_CORES):
        om = np.asarray(res.results[m]["o"])  # [128, 64] f32, o^T
        dm = np.asarray(res.results[m]["den"]).reshape(-1)  # [4*ttot]
        for i, b in enumerate(order):
            off = offs[b]
            nt = nts[b]
            den = dm[HPC * off : HPC * off + HPC * nt].reshape(nt, HPC).sum(axis=0)
            oT = om[:, HPC * i : HPC * i + HPC]  # [128, 4]
            out[b, 0, HPC * m : HPC * m + HPC, :] = (oT / den[None, :]).T
    return out, res


def kernel(**inputs) -> np.ndarray:
    out, _ = _run(inputs, trace=False)
    return out
